# revision 1
# baseline (speedup 1.0000x reference)
"""Trainium2 Bass kernel for DiceLossWithLongLabels (35-label 3D dice loss
with trilinear grid-sample warp of one-hot moving labels).

Strategy (8 NeuronCores, SPMD):
  - Spatially shard the 2M output voxels across cores (16 z_out slices each).
  - Each core holds the full y_moving volume; it builds a DRAM "neighborhood
    table" nb16[q] = the 8 corner labels (bf16) of the 2x2x2 cell based at
    position q.  One 16B indirect-DMA gather per voxel then fetches all 8
    corner labels needed for trilinear interpolation.
  - Per-dim clamped-base + weight-fixup handles zeros-padding boundary
    semantics without any masks downstream.
  - Dice sums are computed as 35-bin fused compare-multiply-accumulate
    passes (scalar_tensor_tensor) over the gathered corner labels:
      inter_c = sum_n [f_n==c] * s_n        (s = warped prob at fixed label)
      fix_c   = sum_n [f_n==c]
      u2_c    = sum_{n,j} [m_j==c] * t_j    (t_j = w_j * W_{m_j})
  - Each core returns per-partition-summed partials; the host sums cores
    and applies the final (2*inter+eps)/(union+eps) formula.
"""

import sys

sys.path.insert(0, "/opt/trn_rl_repo")

from contextlib import ExitStack

import numpy as np

import concourse.bacc as bacc
import concourse.bass as bass
import concourse.mybir as mybir
import concourse.tile as tile
from concourse.tile import add_dep_helper

P = 128
ALU = mybir.AluOpType
DT = mybir.dt
AFT = mybir.ActivationFunctionType


def build_program(VOL=128, ZSL=16, F=512, debug_dump=False):
    """Build the SPMD bass program.

    VOL: volume side length (cube).  ZSL: z_out slices per core.
    F: free-dim elements per partition per chunk (chunk = 128*F voxels).
    """
    YX = VOL * VOL
    NTBL = VOL * YX              # table rows
    N = ZSL * YX                 # output voxels per core
    assert N % (P * F) == 0
    NCHUNK = N // (P * F)
    PER_PART = N // P            # voxels per partition over all chunks
    assert PER_PART % F == 0

    # table build staging: split the per-partition row (YX elems) into pieces
    TSTG = min(2048, YX)
    assert YX % TSTG == 0
    NSTG = YX // TSTG
    # table interleave chunking (yx positions per chunk)
    TF = min(512, YX)
    assert YX % TF == 0
    NTC = YX // TF

    SCALE = VOL / 2.0
    BIAS = VOL / 2.0 - 0.5
    CLIPMAX = float(VOL - 2)

    NB = 35  # labels 1..35

    nc = bacc.Bacc("TRN2", target_bir_lowering=False, debug=False)

    y_moving = nc.dram_tensor("y_moving", [NTBL], DT.int32, kind="ExternalInput")
    warp_sl = nc.dram_tensor("warp_slice", [N, 3], DT.float32, kind="ExternalInput")
    y_fixed = nc.dram_tensor("y_fixed_slice", [N], DT.int32, kind="ExternalInput")
    partials = nc.dram_tensor(
        "partials", [3 * NCHUNK * NB], DT.float32, kind="ExternalOutput"
    )
    nb16 = nc.dram_tensor("nb16", [NTBL, 8], DT.bfloat16)
    if debug_dump:
        dbg_qi = nc.dram_tensor("dbg_qi", [P, F], DT.int32, kind="ExternalOutput")
        dbg_s = nc.dram_tensor("dbg_s", [P, F], DT.float32, kind="ExternalOutput")
        dbg_g = nc.dram_tensor("dbg_g", [P, F * 8], DT.bfloat16, kind="ExternalOutput")
        dbg_w = nc.dram_tensor("dbg_w", [P, F * 8], DT.float32, kind="ExternalOutput")

    ymov_v = y_moving.ap().rearrange("(z yx) -> z yx", z=VOL)  # [VOL, YX]
    nb_zv = nb16.ap().rearrange("(z yx) e -> z (yx e)", z=VOL)
    warp_v = warp_sl.ap().rearrange("(p v) c -> p (v c)", p=P)  # [128, PER_PART*3]
    yfix_v = y_fixed.ap().rearrange("(p v) -> p v", p=P)        # [128, PER_PART]

    table_writes = []

    with tile.TileContext(nc) as tc:
        with ExitStack() as ctx:
            # ---------------- persistent small tiles ----------------
            const_pool = ctx.enter_context(tc.tile_pool(name="const", bufs=1))
            acc_pool = ctx.enter_context(tc.tile_pool(name="acc", bufs=1))

            zeros_f = const_pool.tile([P, F], DT.float32)
            nc.vector.memset(zeros_f[:], 0.0)
            ones_b = const_pool.tile([P, F], DT.bfloat16)
            nc.vector.memset(ones_b[:], 1.0)
            ones_pe = const_pool.tile([P, 1], DT.float32)
            nc.vector.memset(ones_pe[:], 1.0)
            negc = const_pool.tile([P, 35], DT.float32)
            for j in range(35):
                nc.vector.memset(negc[:, j : j + 1], -float(j + 1))

            # accumulators: col layout [metric(3)][chunk][bin]
            NACC = 3 * NCHUNK * NB
            acc = acc_pool.tile([P, NACC], DT.float32)

            def acc_col(metric, chunk, c):
                col = metric * NCHUNK * NB + chunk * NB + (c - 1)
                return acc[:, col : col + 1]

            # ---------------- phase 1: neighborhood table ----------------
            with tc.tile_pool(name="tbl", bufs=1) as tblp, tc.tile_pool(
                name="tblo", bufs=3
            ) as tbop:
                # Lcat: per partition z (z < VOL): [0:YX) = labels at z,
                # [YX:2YX) = labels at z+1
                pad = 2 * TF
                lcat = tblp.tile([P, 2 * YX + pad], DT.bfloat16)
                nc.gpsimd.memset(lcat[:], 0.0)

                def load_cast(dst_cols, src_ap, p0, p1, role):
                    # SWDGE DMA casts int32 -> bf16 inline during transfer
                    nc.gpsimd.dma_start(
                        lcat[p0:p1, dst_cols[0] : dst_cols[1]], src_ap
                    )

                for r in range(NSTG):
                    cs = r * TSTG
                    # slice a: partition z <- z
                    load_cast(
                        (cs, cs + TSTG), ymov_v[:, cs : cs + TSTG], 0, VOL, "a"
                    )
                    # slice b: partition z <- z+1 (z<VOL-1); z=VOL-1 stays at
                    # the memset value (those table rows are never gathered)
                    load_cast(
                        (YX + cs, YX + cs + TSTG),
                        ymov_v[1:VOL, cs : cs + TSTG],
                        0,
                        VOL - 1,
                        "b",
                    )

                # interleave 8 shifted copies -> nb rows, chunk by chunk
                engines = [nc.vector, nc.scalar]
                for t in range(NTC):
                    off = t * TF
                    nbt = tbop.tile([P, TF * 8], DT.bfloat16, tag=f"nbt{t % 2}")
                    nbv = nbt[:].rearrange("p (f k) -> p f k", k=8)
                    for dz in (0, 1):
                        for dy in (0, 1):
                            for dx in (0, 1):
                                k = dz * 4 + dy * 2 + dx
                                src_off = off + dz * YX + dy * VOL + dx
                                eng = engines[t % 2]
                                if eng is nc.scalar:
                                    eng.copy(
                                        out=nbv[:VOL, :, k],
                                        in_=lcat[:VOL, src_off : src_off + TF],
                                    )
                                else:
                                    eng.tensor_copy(
                                        out=nbv[:VOL, :, k],
                                        in_=lcat[:VOL, src_off : src_off + TF],
                                    )
                    w = nc.sync.dma_start(
                        out=nb_zv[:, off * 8 : (off + TF) * 8], in_=nbt[:VOL, :]
                    )
                    table_writes.append(w)

            # ---------------- phase 2: main voxel loop ----------------
            io_pool = ctx.enter_context(tc.tile_pool(name="io", bufs=2))
            cs_pool = ctx.enter_context(tc.tile_pool(name="csc", bufs=1))
            g_pool = ctx.enter_context(tc.tile_pool(name="gg", bufs=2))
            big_pool = ctx.enter_context(tc.tile_pool(name="big", bufs=1))
            sc_pool = ctx.enter_context(tc.tile_pool(name="scr", bufs=1))

            for ch in range(NCHUNK):
                wt = io_pool.tile([P, 3 * F], DT.float32, tag="warp")
                nc.sync.dma_start(
                    wt[:], warp_v[:, ch * 3 * F : (ch + 1) * 3 * F]
                )
                ffi = io_pool.tile([P, F], DT.int32, tag="ffi")
                nc.sync.dma_start(ffi[:], yfix_v[:, ch * F : (ch + 1) * F])
                fff = g_pool.tile([P, F], DT.float32, tag="fff")
                nc.vector.tensor_copy(out=fff[:], in_=ffi[:])
                ffb = g_pool.tile([P, F], DT.bfloat16, tag="ffb")
                nc.vector.tensor_copy(out=ffb[:], in_=fff[:])

                # --- coordinates per dim: d=0 x, 1 y, 2 z ---
                wa = {}
                wb = {}
                qacc = None
                for d in (2, 1, 0):  # z, y, x (q accumulation order)
                    gv = wt[:, d :: 3]
                    it = cs_pool.tile([P, F], DT.float32, tag="tmp_i")
                    nc.scalar.activation(
                        out=it[:], in_=gv, func=AFT.Copy, bias=BIAS, scale=SCALE
                    )
                    ri = cs_pool.tile([P, F], DT.int32, tag="ri")
                    nc.vector.tensor_copy(out=ri[:], in_=it[:])
                    rf = cs_pool.tile([P, F], DT.float32, tag="rf")
                    nc.vector.tensor_copy(out=rf[:], in_=ri[:])
                    gt = cs_pool.tile([P, F], DT.float32, tag="gt")
                    nc.vector.tensor_tensor(
                        out=gt[:], in0=rf[:], in1=it[:], op=ALU.is_gt
                    )
                    bf = cs_pool.tile([P, F], DT.float32, tag="tmp_b")
                    nc.vector.tensor_tensor(
                        out=bf[:], in0=rf[:], in1=gt[:], op=ALU.subtract
                    )
                    fr = cs_pool.tile([P, F], DT.float32, tag="tmp_f")
                    nc.vector.tensor_tensor(
                        out=fr[:], in0=it[:], in1=bf[:], op=ALU.subtract
                    )
                    omf = cs_pool.tile([P, F], DT.float32, tag="tmp_o")
                    nc.scalar.activation(
                        out=omf[:], in_=fr[:], func=AFT.Copy, bias=1.0, scale=-1.0
                    )
                    mneg = cs_pool.tile([P, F], DT.int8, tag="mneg")
                    nc.vector.tensor_scalar(
                        out=mneg[:], in0=bf[:], scalar1=0.0, scalar2=None,
                        op0=ALU.is_lt,
                    )
                    mhi = cs_pool.tile([P, F], DT.int8, tag="mhi")
                    nc.vector.tensor_scalar(
                        out=mhi[:], in0=bf[:], scalar1=CLIPMAX, scalar2=None,
                        op0=ALU.is_gt,
                    )
                    wad = cs_pool.tile([P, F], DT.float32, tag=f"wa{d}")
                    nc.vector.tensor_copy(out=wad[:], in_=omf[:])
                    nc.vector.copy_predicated(out=wad[:], mask=mhi[:], data=zeros_f[:])
                    nc.vector.copy_predicated(out=wad[:], mask=mneg[:], data=fr[:])
                    wbd = cs_pool.tile([P, F], DT.float32, tag=f"wb{d}")
                    nc.vector.tensor_copy(out=wbd[:], in_=fr[:])
                    nc.vector.copy_predicated(out=wbd[:], mask=mhi[:], data=omf[:])
                    nc.vector.copy_predicated(out=wbd[:], mask=mneg[:], data=zeros_f[:])
                    wa[d] = wad
                    wb[d] = wbd
                    # clip base and accumulate q
                    bcl = cs_pool.tile([P, F], DT.float32, tag="bcl")
                    nc.vector.tensor_scalar(
                        out=bcl[:], in0=bf[:], scalar1=0.0, scalar2=CLIPMAX,
                        op0=ALU.max, op1=ALU.min,
                    )
                    if qacc is None:
                        qacc = cs_pool.tile([P, F], DT.float32, tag="qacc")
                        nc.vector.tensor_copy(out=qacc[:], in_=bcl[:])
                    else:
                        nc.vector.scalar_tensor_tensor(
                            out=qacc[:], in0=qacc[:], scalar=float(VOL),
                            in1=bcl[:], op0=ALU.mult, op1=ALU.add,
                        )
                qi = cs_pool.tile([P, F], DT.int32, tag="qi")
                nc.vector.tensor_copy(out=qi[:], in_=qacc[:])

                # --- corner weights w8 (interleaved, bf16) ---
                w8 = big_pool.tile([P, F * 8], DT.float32, tag="w8")
                w8v = w8[:].rearrange("p (f k) -> p f k", k=8)
                for dy in (0, 1):
                    for dx in (0, 1):
                        wxy = cs_pool.tile([P, F], DT.float32, tag="wxy")
                        nc.vector.tensor_tensor(
                            out=wxy[:],
                            in0=(wa[1] if dy == 0 else wb[1])[:],
                            in1=(wa[0] if dx == 0 else wb[0])[:],
                            op=ALU.mult,
                        )
                        for dz in (0, 1):
                            k = dz * 4 + dy * 2 + dx
                            nc.vector.tensor_tensor(
                                out=w8v[:, :, k],
                                in0=wxy[:],
                                in1=(wa[2] if dz == 0 else wb[2])[:],
                                op=ALU.mult,
                            )

                # --- gather 8 corner labels ---
                G = g_pool.tile([P, F * 8], DT.bfloat16, tag="G")
                gather = nc.gpsimd.indirect_dma_start(
                    out=G[:],
                    out_offset=None,
                    in_=nb16.ap(),
                    in_offset=bass.IndirectOffsetOnAxis(ap=qi[:], axis=0),
                )
                for w in table_writes:
                    add_dep_helper(
                        gather.ins, w.ins, reason="table before gather"
                    )
                Gv = G[:].rearrange("p (f k) -> p f k", k=8)

                # --- s = warped prob at fixed label ---
                E8 = big_pool.tile([P, F * 8], DT.float32, tag="E8")
                E8v = E8[:].rearrange("p (f k) -> p f k", k=8)
                ffb3 = ffb[:].rearrange("p (f o) -> p f o", o=1).to_broadcast(
                    [P, F, 8]
                )
                nc.vector.tensor_tensor(out=E8v, in0=Gv, in1=ffb3, op=ALU.is_equal)
                nc.vector.tensor_tensor(out=E8v, in0=E8v, in1=w8v, op=ALU.mult)
                s_t = g_pool.tile([P, F], DT.float32, tag="s_t")
                nc.vector.tensor_reduce(
                    out=s_t[:], in_=E8v, axis=mybir.AxisListType.X, op=ALU.add
                )

                # --- t_j = w_j * W_{m_j} ---
                T8 = big_pool.tile([P, F * 8], DT.float32, tag="T8")
                T8v = T8[:].rearrange("p (f k) -> p f k", k=8)
                for j in range(8):
                    gjb = Gv[:, :, j : j + 1].to_broadcast([P, F, 8])
                    nc.vector.tensor_tensor(
                        out=E8v, in0=Gv, in1=gjb, op=ALU.is_equal
                    )
                    nc.vector.tensor_tensor(out=E8v, in0=E8v, in1=w8v, op=ALU.mult)
                    wm = cs_pool.tile([P, F], DT.float32, tag="wm")
                    nc.vector.tensor_reduce(
                        out=wm[:], in_=E8v, axis=mybir.AxisListType.X, op=ALU.add
                    )
                    nc.vector.tensor_tensor(
                        out=T8v[:, :, j], in0=wm[:], in1=w8v[:, :, j], op=ALU.mult
                    )

                if debug_dump and ch == 0:
                    nc.sync.dma_start(dbg_qi.ap(), qi[:])
                    nc.sync.dma_start(dbg_s.ap(), s_t[:])
                    nc.sync.dma_start(dbg_g.ap(), G[:])
                    nc.sync.dma_start(dbg_w.ap(), w8[:])

                # --- binning ---
                jnk = sc_pool.tile([P, F], DT.float32, tag="jnk")
                jnka = sc_pool.tile([P, F], DT.float32, tag="jnka")
                jnkb = sc_pool.tile([P, F], DT.float32, tag="jnkb")
                jnk8 = E8
                for c in range(1, NB + 1):
                    # inter (vector)
                    nc.vector.scalar_tensor_tensor(
                        out=jnk[:], in0=fff[:], scalar=float(c), in1=s_t[:],
                        op0=ALU.is_equal, op1=ALU.mult,
                        accum_out=acc_col(0, ch, c),
                    )
                    # fix count on ACT: [f==c] = Relu(1 - |f - c|)
                    nc.scalar.activation(
                        out=jnka[:], in_=fff[:], func=AFT.Abs,
                        bias=negc[:, c - 1 : c], scale=1.0,
                    )
                    nc.scalar.activation(
                        out=jnkb[:], in_=jnka[:], func=AFT.Relu,
                        bias=1.0, scale=-1.0, accum_out=acc_col(1, ch, c),
                    )
                    # u2 (vector)
                    nc.vector.scalar_tensor_tensor(
                        out=jnk8[:], in0=G[:], scalar=float(c), in1=T8[:],
                        op0=ALU.is_equal, op1=ALU.mult,
                        accum_out=acc_col(2, ch, c),
                    )

            # ---------------- final cross-partition reduce ----------------
            with tc.tile_pool(name="ps", bufs=1, space="PSUM") as psp:
                pst = psp.tile([P, NACC], DT.float32, space="PSUM")
                nc.tensor.matmul(
                    out=pst[0:1, :],
                    lhsT=ones_pe[:],
                    rhs=acc[:],
                    start=True,
                    stop=True,
                )
                outt = const_pool.tile([1, NACC], DT.float32)
                nc.vector.tensor_copy(out=outt[:], in_=pst[0:1, :])
                nc.sync.dma_start(out=partials.ap(), in_=outt[:])

    nc.compile()
    return nc, dict(VOL=VOL, ZSL=ZSL, F=F, NCHUNK=NCHUNK, NB=NB)


_CACHE = {}


def _get_program():
    if "prog" not in _CACHE:
        _CACHE["prog"] = build_program()
    return _CACHE["prog"]


def _host_combine(partial_list, meta):
    NCHUNK, NB = meta["NCHUNK"], meta["NB"]
    tot = np.zeros(3 * NCHUNK * NB, dtype=np.float64)
    for pa in partial_list:
        tot += np.asarray(pa, dtype=np.float64)
    m = tot.reshape(3, NCHUNK, NB).sum(axis=1)  # [3, 35]
    inter, fix, u2 = m[0], m[1], m[2]
    eps = 1e-6
    union = fix + u2
    dice = (2.0 * inter + eps) / (union + eps)
    return (1.0 - dice).astype(np.float32)


def kernel(y_moving, y_fixed, warp):
    from concourse.bass_utils import run_bass_kernel_spmd

    nc, meta = _get_program()
    VOL, ZSL = meta["VOL"], meta["ZSL"]
    ncores = VOL // ZSL
    ym = np.ascontiguousarray(y_moving[0, 0].reshape(-1).astype(np.int32))
    in_maps = []
    for c in range(ncores):
        z0, z1 = c * ZSL, (c + 1) * ZSL
        in_maps.append(
            {
                "y_moving": ym,
                "warp_slice": np.ascontiguousarray(
                    warp[0, z0:z1].reshape(-1, 3).astype(np.float32)
                ),
                "y_fixed_slice": np.ascontiguousarray(
                    y_fixed[0, 0, z0:z1].reshape(-1).astype(np.int32)
                ),
            }
        )
    res = run_bass_kernel_spmd(nc, in_maps, list(range(ncores)))
    partial_list = [res.results[i]["partials"] for i in range(ncores)]
    return _host_combine(partial_list, meta)

